# revision 1
# baseline (speedup 1.0000x reference)
"""MoE SwiGLU experts kernel for Trainium2, 8 NeuronCores — oct F-split.

Every core processes ALL 8 experts over its own F/8 = 512 slice of the
FFN dimension (wg/wu columns, wd rows).  Token counts per expert are
processed exactly (no global-max padding), so PE columns hit the floor.
The host sums the 8 partial down-projections.  fp16 operands, fp32 PSUM.

All DRAM tensors are partition-major ([128, ...free]) so whole-expert
transfers are single DMAs with 2-16KB descriptor runs; total DMA count
is ~40 (HWDGE issue is ~630ns per DMA and was the bottleneck of the
first oct attempt at ~350 DMAs).
"""

import math

import numpy as np

import concourse.bass as bass
import concourse.mybir as mybir
import concourse.tile as tile
from concourse import bass2jax
from concourse.bass_utils import run_bass_kernel_spmd

import orjson

FP32 = mybir.dt.float32
FP16 = mybir.dt.float16

D = 1024        # model dim
F = 4096        # ffn dim
FQ = F // 8     # per-core F slice
E = 8           # experts
KD = D // 128   # 8  k-tiles over D
KF = FQ // 128  # 4  f-tiles over the F slice


# ---------------------------------------------------------------------------
# BIR legalizer: this container's walrus accepts at most ONE sync-wait per
# instruction.  Tile emits instructions with several waits; hoist the excess
# onto preceding EventSemaphore carrier instructions on the same engine
# (engines execute their stream in order, so waiting earlier is equivalent).
# ---------------------------------------------------------------------------

def _legalize_bir_waits(bir_bytes: bytes) -> bytes:
    bir = orjson.loads(bir_bytes)
    n_fix = 0
    for f in bir.get("functions", []):
        for b in f.get("blocks", []):
            out = []
            for inst in b.get("instructions", []):
                si = inst.get("sync_info")
                waits = (si or {}).get("on_wait") or []
                if len(waits) > 1:
                    keep = waits[-1:]
                    excess = waits[:-1]
                    for w in excess:
                        n_fix += 1
                        out.append({
                            "debug": inst.get("debug", 0),
                            "engine": inst["engine"],
                            "ins": [], "outs": [],
                            "name": f"wfix-{n_fix}-{inst['name']}",
                            "opcode": "EventSemaphore",
                            "sync_info": {"on_update": [], "on_wait": [w]},
                        })
                    si["on_wait"] = keep
                out.append(inst)
            b["instructions"] = out
    return orjson.dumps(bir)


_orig_decompress = bass2jax._decompress_ant_bir


def _patched_decompress(v):
    return _legalize_bir_waits(_orig_decompress(v))


bass2jax._decompress_ant_bir = _patched_decompress




def _chunks(cap, tail_first=True):
    out = []
    c0 = 0
    while cap - c0 >= 512:
        out.append((c0, 512))
        c0 += 512
    if cap - c0 > 0:
        tail = (c0, cap - c0)
        out = [tail] + out if tail_first else out + [tail]
    return out


def build_program(caps, reps: int = 1) -> bass.Bass:
    """caps: per-expert padded token counts (len 8, multiples of 8)."""
    caps = list(caps)
    T = sum(caps)
    starts = [0]
    for c in caps:
        starts.append(starts[-1] + c)

    nc = bass.Bass()
    # p-major layouts: x[p, k, t]; wgu[p, (e), fj, m, k, f]; wd[p, (e), d, fj, dd]
    x_d = nc.declare_dram_parameter("x", [128, KD, T], FP16, isOutput=False)
    wgu_d = nc.declare_dram_parameter(
        "wgu", [E, 128, KF, 2, KD, 128], FP16, isOutput=False)
    wd_d = nc.declare_dram_parameter(
        "wd", [E, 128, KD, KF, 128], FP16, isOutput=False)
    y_d = nc.declare_dram_parameter("y", [128, KD, T], FP16, isOutput=True)

    with tile.TileContext(nc) as tc:
        with (
            tc.tile_pool(name="xp", bufs=2) as xp,
            tc.tile_pool(name="hp", bufs=2) as hp,
            tc.tile_pool(name="w1p", bufs=2) as w1p,
            tc.tile_pool(name="w2p", bufs=2) as w2p,
            tc.tile_pool(name="sp", bufs=2) as sp,
            tc.tile_pool(name="yp", bufs=2) as yp,
            tc.tile_pool(name="pg", bufs=2, space=bass.MemorySpace.PSUM) as pg,
            tc.tile_pool(name="pu", bufs=2, space=bass.MemorySpace.PSUM) as pu,
            tc.tile_pool(name="py", bufs=4, space=bass.MemorySpace.PSUM) as py,
        ):
            for _ in range(reps):
                x_t, wgu_t, wd_t = {}, {}, {}

                # All DMA issue goes through the SP queue: the ACT queue's
                # sequencer is blocked by in-flight silu waits, and transfer
                # order follows issue order — so SP program order IS the
                # transfer priority.
                def fetch_x(e, split):
                    c = caps[e]
                    s = starts[e]
                    xt = xp.tile([128, KD, c], FP16, tag="x", name=f"x{e}")
                    if split:
                        for k in range(KD):
                            nc.sync.dma_start(xt[:, k, :], x_d[:, k, s:s + c])
                    else:
                        nc.sync.dma_start(xt[:], x_d[:, :, s:s + c])
                    x_t[e] = xt

                def fetch_wgu(e, split, lo=0, hi=KF):
                    if lo == 0:
                        wgu_t[e] = w1p.tile([128, KF, 2, KD, 128], FP16,
                                            tag="wgu", name=f"wgu{e}")
                    wt = wgu_t[e]
                    if split:
                        for fj in range(lo, hi):
                            nc.sync.dma_start(wt[:, fj], wgu_d[e][:, fj])
                    else:
                        nc.sync.dma_start(wt[:], wgu_d[e])

                def fetch_wd(e):
                    wt = w2p.tile([128, KD, KF, 128], FP16, tag="wd",
                                  name=f"wd{e}")
                    nc.sync.dma_start(wt[:], wd_d[e])
                    wd_t[e] = wt

                # lead-in priority: first f-tile pair, then x, then the rest
                fetch_wgu(0, split=True, lo=0, hi=1)
                fetch_x(0, split=True)
                fetch_wgu(0, split=True, lo=1, hi=KF)
                fetch_wd(0)

                for e in range(E):
                    cap = caps[e]
                    s0 = starts[e]
                    chunks = _chunks(cap)
                    if e + 1 < E:
                        fetch_wgu(e + 1, split=False)
                        fetch_x(e + 1, split=False)
                        fetch_wd(e + 1)

                    xt = x_t.pop(e)
                    wgu = wgu_t.pop(e)  # noqa: assigned in fetch_wgu
                    h = hp.tile([128, KF, cap], FP16, tag="h", name=f"h{e}")

                    # ---- phase 1: G/U; H = silu(G) * U ----
                    for fj in range(KF):
                        for (c0, w) in chunks:
                            g_ps = pg.tile([128, 512], FP32, tag="g",
                                           name="g_ps")
                            for k in range(KD):
                                nc.tensor.matmul(
                                    g_ps[:, :w],
                                    wgu[:, fj, 0, k, :], xt[:, k, c0:c0 + w],
                                    start=(k == 0), stop=(k == KD - 1),
                                )
                            u_ps = pu.tile([128, 512], FP32, tag="u",
                                           name="u_ps")
                            for k in range(KD):
                                nc.tensor.matmul(
                                    u_ps[:, :w],
                                    wgu[:, fj, 1, k, :], xt[:, k, c0:c0 + w],
                                    start=(k == 0), stop=(k == KD - 1),
                                )
                            sg = sp.tile([128, 512], FP16, tag="sg",
                                         name="sg")
                            nc.scalar.activation(
                                sg[:, :w], g_ps[:, :w],
                                mybir.ActivationFunctionType.Silu,
                            )
                            nc.vector.tensor_mul(
                                h[:, fj, c0:c0 + w], sg[:, :w], u_ps[:, :w]
                            )

                    # ---- phase 2: Y partial = Wd^T @ H (accum over f) ----
                    wdt = wd_t.pop(e)
                    y_all = yp.tile([128, KD, cap], FP16, tag="y",
                                    name=f"y{e}")
                    last_e = e == E - 1
                    for d in range(KD):
                        d_chunks = chunks if not (last_e and d == KD - 1) \
                            else sorted(chunks, key=lambda cw: -cw[1])
                        for (c0, w) in d_chunks:
                            y_ps = py.tile([128, 512], FP32, tag="yp",
                                           name="y_ps")
                            for fj in range(KF):
                                nc.tensor.matmul(
                                    y_ps[:, :w],
                                    wdt[:, d, fj, :], h[:, fj, c0:c0 + w],
                                    start=(fj == 0), stop=(fj == KF - 1),
                                )
                            nc.vector.tensor_copy(
                                y_all[:, d, c0:c0 + w], y_ps[:, :w]
                            )
                            if last_e and d == KD - 1:
                                # drain the final d-tile chunk-wise on
                                # alternating queues (ACT is silu-free here)
                                y_eng = nc.scalar if c0 == 0 else nc.sync
                                y_eng.dma_start(
                                    y_d[:, d, s0 + c0:s0 + c0 + w],
                                    y_all[:, d, c0:c0 + w],
                                )
                        if last_e and d < KD - 1:
                            # stream the tail expert out per d-tile
                            nc.sync.dma_start(
                                y_d[:, d, s0:s0 + cap], y_all[:, d, :]
                            )
                    if not last_e:
                        nc.sync.dma_start(
                            y_d[:, :, s0:s0 + cap], y_all[:]
                        )

    return nc


# ---------------------------------------------------------------------------
# Host-side sharding / unsharding
# ---------------------------------------------------------------------------

def _prep_wgu(wg16, wu16):
    """two [D, FQ] fp16 -> [128, KF, 2, KD, 128] p-major."""
    def t(w):
        return w.reshape(KD, 128, KF, 128).transpose(1, 2, 0, 3)  # p,fj,k,f
    return np.ascontiguousarray(
        np.stack([t(wg16), t(wu16)], axis=2))       # p, fj, m, k, f


def _prep_wd(w16):
    """[FQ, D] fp16 -> [128, KD, KF, 128] p-major."""
    arr = w16.reshape(KF, 128, KD, 128)             # fj, p, d, dd
    return np.ascontiguousarray(arr.transpose(1, 2, 0, 3))


_prog_cache = {}


def get_program(caps):
    key = tuple(caps)
    if key not in _prog_cache:
        _prog_cache[key] = build_program(caps)
    return _prog_cache[key]


def prepare_in_maps(np_inputs):
    x = np.asarray(np_inputs["x"])
    B, S, _ = x.shape
    xf = np.asarray(x.reshape(-1, D), dtype=np.float16)
    idx = np.asarray(np_inputs["expert_idx"]).reshape(-1)
    w_gate = np.asarray(np_inputs["w_gate"], dtype=np.float16)
    w_up = np.asarray(np_inputs["w_up"], dtype=np.float16)
    w_down = np.asarray(np_inputs["w_down"], dtype=np.float16)

    order = np.argsort(idx, kind="stable")
    counts = np.bincount(idx, minlength=E).astype(np.int64)
    caps = [max(16, int(math.ceil(max(c, 1) / 8)) * 8) for c in counts]
    T = sum(caps)
    starts = np.zeros(E + 1, dtype=np.int64)
    np.cumsum(caps, out=starts[1:])
    cstarts = np.zeros(E + 1, dtype=np.int64)
    np.cumsum(counts, out=cstarts[1:])

    tok_of = [order[cstarts[e]:cstarts[e + 1]] for e in range(E)]

    # x packed [128, KD, T] p-major, per-expert segments padded to caps
    xt = np.zeros((D, T), dtype=np.float16)
    for e in range(E):
        xt[:, starts[e]:starts[e] + counts[e]] = xf[tok_of[e]].T
    x_arr = np.ascontiguousarray(
        xt.reshape(KD, 128, T).transpose(1, 0, 2))

    in_maps = []
    for q in range(E):
        sl = slice(q * FQ, (q + 1) * FQ)
        wgu = np.ascontiguousarray(np.stack(
            [_prep_wgu(w_gate[e][:, sl], w_up[e][:, sl]) for e in range(E)]))
        wd = np.ascontiguousarray(np.stack(
            [_prep_wd(w_down[e][sl, :]) for e in range(E)]))
        in_maps.append({"x": x_arr, "wgu": wgu, "wd": wd})
    meta = (tok_of, counts, starts, (B, S), x.dtype)
    return in_maps, caps, meta


def unshard(results, caps, meta):
    tok_of, counts, starts, (B, S), out_dtype = meta
    T = sum(caps)
    acc = np.zeros((128, KD, T), dtype=np.float32)
    for q in range(E):
        acc += np.asarray(results[q]["y"]).astype(np.float32)
    acc = acc.transpose(1, 0, 2).reshape(D, T)
    out = np.zeros((B * S, D), dtype=np.float32)
    for e in range(E):
        out[tok_of[e]] = acc[:, starts[e]:starts[e] + counts[e]].T
    return out.reshape(B, S, D).astype(out_dtype, copy=False)


def kernel(x, expert_idx, w_gate, w_up, w_down):
    np_inputs = {"x": x, "expert_idx": expert_idx, "w_gate": w_gate,
                 "w_up": w_up, "w_down": w_down}
    in_maps, caps, meta = prepare_in_maps(np_inputs)
    nc = get_program(caps)
    try:
        res = run_bass_kernel_spmd(nc, in_maps, list(range(8)))
    except Exception:
        res = run_bass_kernel_spmd(nc, in_maps, list(range(8)))
    return unshard(res.results, caps, meta)



# revision 5
# speedup vs baseline: 1.0603x; 1.0603x over previous
"""MoE SwiGLU experts kernel for Trainium2, 8 NeuronCores — oct F-split.

Every core processes ALL 8 experts over its own F/8 = 512 slice of the
FFN dimension (wg/wu columns, wd rows).  Token counts per expert are
processed exactly (no global-max padding), so PE columns hit the floor.
The host sums the 8 partial down-projections.  fp16 operands, fp32 PSUM.

All DRAM tensors are partition-major ([128, ...free]) so whole-expert
transfers are single DMAs with 2-16KB descriptor runs; total DMA count
is ~40 (HWDGE issue is ~630ns per DMA and was the bottleneck of the
first oct attempt at ~350 DMAs).
"""

import math

import numpy as np

import concourse.bass as bass
import concourse.mybir as mybir
import concourse.tile as tile
from concourse import bass2jax
from concourse.bass_utils import run_bass_kernel_spmd

import orjson

FP32 = mybir.dt.float32
FP16 = mybir.dt.float16

D = 1024        # model dim
F = 4096        # ffn dim
FQ = F // 8     # per-core F slice
E = 8           # experts
KD = D // 128   # 8  k-tiles over D
KF = FQ // 128  # 4  f-tiles over the F slice


# ---------------------------------------------------------------------------
# BIR legalizer: this container's walrus accepts at most ONE sync-wait per
# instruction.  Tile emits instructions with several waits; hoist the excess
# onto preceding EventSemaphore carrier instructions on the same engine
# (engines execute their stream in order, so waiting earlier is equivalent).
# ---------------------------------------------------------------------------

def _dedupe_ldweights(bir: dict) -> None:
    """Remove Ldweights whose operand AP is identical to the immediately
    preceding Ldweights on the PE stream (weights already resident in the
    array).  The build emits matmuls k-outer/chunk-inner, so every weight
    tile is loaded once and reused by 2-3 moving chunks; the redundant
    reloads otherwise cost ~53ns of PE array time each (unmodeled by the
    cost model, visible on HW).  Sync info of a removed Ldweights is
    preserved on an EventSemaphore carrier in its place."""
    n_drop = 0
    for f in bir.get("functions", []):
        for b in f.get("blocks", []):
            out = []
            last_key = None
            for inst in b.get("instructions", []):
                op = inst.get("opcode")
                if op == "Ldweights":
                    key = orjson.dumps(
                        (inst.get("ins"), inst.get("perf_mode"),
                         inst.get("is_transpose"), inst.get("tile_position"))
                    )
                    if key == last_key:
                        n_drop += 1
                        si = inst.get("sync_info") or {}
                        if si.get("on_wait") or si.get("on_update"):
                            out.append({
                                "debug": inst.get("debug", 0),
                                "engine": inst["engine"],
                                "ins": [], "outs": [],
                                "name": f"ldwdrop-{n_drop}-{inst['name']}",
                                "opcode": "EventSemaphore",
                                "sync_info": si,
                            })
                        continue
                    last_key = key
                elif op == "Matmult":
                    pass  # matmuls keep the resident weights
                else:
                    # any other PE instruction may clobber array state;
                    # only same-block, same-stream reuse is assumed
                    if inst.get("engine") in ("PE", 4):
                        last_key = None
                out.append(inst)
            b["instructions"] = out


def _legalize_bir_waits(bir_bytes: bytes) -> bytes:
    bir = orjson.loads(bir_bytes)
    _dedupe_ldweights(bir)
    n_fix = 0
    for f in bir.get("functions", []):
        for b in f.get("blocks", []):
            out = []
            for inst in b.get("instructions", []):
                si = inst.get("sync_info")
                waits = (si or {}).get("on_wait") or []
                if len(waits) > 1:
                    keep = waits[-1:]
                    excess = waits[:-1]
                    for w in excess:
                        n_fix += 1
                        out.append({
                            "debug": inst.get("debug", 0),
                            "engine": inst["engine"],
                            "ins": [], "outs": [],
                            "name": f"wfix-{n_fix}-{inst['name']}",
                            "opcode": "EventSemaphore",
                            "sync_info": {"on_update": [], "on_wait": [w]},
                        })
                    si["on_wait"] = keep
                out.append(inst)
            b["instructions"] = out
    return orjson.dumps(bir)


_orig_decompress = bass2jax._decompress_ant_bir


def _patched_decompress(v):
    return _legalize_bir_waits(_orig_decompress(v))


bass2jax._decompress_ant_bir = _patched_decompress




def _chunks(cap, tail_first=True):
    out = []
    c0 = 0
    while cap - c0 >= 512:
        out.append((c0, 512))
        c0 += 512
    if cap - c0 > 0:
        tail = (c0, cap - c0)
        out = [tail] + out if tail_first else out + [tail]
    return out


def build_program(caps, reps: int = 1) -> bass.Bass:
    """caps: per-expert padded token counts (len 8, multiples of 8)."""
    caps = list(caps)
    T = sum(caps)
    starts = [0]
    for c in caps:
        starts.append(starts[-1] + c)

    nc = bass.Bass()
    # p-major layouts: x[p, k, t]; wgu[p, (e), fj, m, k, f]; wd[p, (e), d, fj, dd]
    x_d = nc.declare_dram_parameter("x", [128, KD, T], FP16, isOutput=False)
    wgu_d = nc.declare_dram_parameter(
        "wgu", [E, 128, KF, 2, KD, 128], FP16, isOutput=False)
    wd_d = nc.declare_dram_parameter(
        "wd", [E, 128, KD, KF, 128], FP16, isOutput=False)
    y_d = nc.declare_dram_parameter("y", [128, KD, T], FP16, isOutput=True)

    with tile.TileContext(nc) as tc:
        with (
            tc.tile_pool(name="xp", bufs=2) as xp,
            tc.tile_pool(name="hp", bufs=2) as hp,
            tc.tile_pool(name="w1p", bufs=2) as w1p,
            tc.tile_pool(name="w2p", bufs=2) as w2p,
            tc.tile_pool(name="sp", bufs=3) as sp,
            tc.tile_pool(name="yp", bufs=2) as yp,
            tc.tile_pool(name="pg", bufs=3, space=bass.MemorySpace.PSUM) as pg,
            tc.tile_pool(name="pu", bufs=3, space=bass.MemorySpace.PSUM) as pu,
            tc.tile_pool(name="py", bufs=2, space=bass.MemorySpace.PSUM) as py,
        ):
            for _ in range(reps):
                x_t, wgu_t, wd_t = {}, {}, {}

                # All DMA issue goes through the SP queue: the ACT queue's
                # sequencer is blocked by in-flight silu waits, and transfer
                # order follows issue order — so SP program order IS the
                # transfer priority.
                def fetch_x(e, split):
                    c = caps[e]
                    s = starts[e]
                    xt = xp.tile([128, KD, c], FP16, tag="x", name=f"x{e}")
                    if split:
                        for k in range(KD):
                            nc.sync.dma_start(xt[:, k, :], x_d[:, k, s:s + c])
                    else:
                        nc.sync.dma_start(xt[:], x_d[:, :, s:s + c])
                    x_t[e] = xt

                def fetch_wgu(e, split, lo=0, hi=KF):
                    if lo == 0:
                        wgu_t[e] = w1p.tile([128, KF, 2, KD, 128], FP16,
                                            tag="wgu", name=f"wgu{e}")
                    wt = wgu_t[e]
                    if split:
                        for fj in range(lo, hi):
                            nc.sync.dma_start(wt[:, fj], wgu_d[e][:, fj])
                    else:
                        nc.sync.dma_start(wt[:], wgu_d[e])

                def fetch_wd(e):
                    wt = w2p.tile([128, KD, KF, 128], FP16, tag="wd",
                                  name=f"wd{e}")
                    nc.sync.dma_start(wt[:], wd_d[e])
                    wd_t[e] = wt

                # lead-in priority: first f-tile pair, then x, then the rest
                fetch_wgu(0, split=True, lo=0, hi=1)
                fetch_x(0, split=True)
                fetch_wgu(0, split=True, lo=1, hi=KF)
                fetch_wd(0)

                for e in range(E):
                    cap = caps[e]
                    s0 = starts[e]
                    chunks = _chunks(cap)
                    if e + 1 < E:
                        fetch_wgu(e + 1, split=False)
                        fetch_x(e + 1, split=False)
                        fetch_wd(e + 1)

                    xt = x_t.pop(e)
                    wgu = wgu_t.pop(e)  # noqa: assigned in fetch_wgu
                    h = hp.tile([128, KF, cap], FP16, tag="h", name=f"h{e}")

                    # ---- phase 1: G/U; H = silu(G) * U ----
                    # k-outer, chunk-inner: each weight tile is loaded into
                    # the PE array once and consumed by every moving chunk
                    # (the BIR pass drops the duplicate Ldweights).  All
                    # chunks accumulate concurrently in separate PSUM banks.
                    for fj in range(KF):
                        gu_ps = []
                        for m, pool in ((0, pg), (1, pu)):
                            pss = [pool.tile([128, 512], FP32, tag=f"gu{m}",
                                             name=f"gu{m}_ps")
                                   for _ in chunks]
                            for k in range(KD):
                                for ps, (c0, w) in zip(pss, chunks):
                                    nc.tensor.matmul(
                                        ps[:, :w],
                                        wgu[:, fj, m, k, :],
                                        xt[:, k, c0:c0 + w],
                                        start=(k == 0), stop=(k == KD - 1),
                                    )
                            gu_ps.append(pss)
                        for ci, (c0, w) in enumerate(chunks):
                            sg = sp.tile([128, 512], FP16, tag="sg",
                                         name="sg")
                            nc.scalar.activation(
                                sg[:, :w], gu_ps[0][ci][:, :w],
                                mybir.ActivationFunctionType.Silu,
                            )
                            nc.vector.tensor_mul(
                                h[:, fj, c0:c0 + w], sg[:, :w],
                                gu_ps[1][ci][:, :w]
                            )

                    # ---- phase 2: Y partial = Wd^T @ H (accum over f) ----
                    # fj-outer within chunk pairs; py holds 2 banks, so
                    # chunks are processed in groups of <=2 per d-tile.
                    wdt = wd_t.pop(e)
                    y_all = yp.tile([128, KD, cap], FP16, tag="y",
                                    name=f"y{e}")
                    last_e = e == E - 1
                    groups = [chunks[i:i + 2] for i in range(0, len(chunks), 2)]
                    for d in range(KD):
                        for grp in groups:
                            pss = [py.tile([128, 512], FP32, tag="yp",
                                           name="y_ps") for _ in grp]
                            for fj in range(KF):
                                for ps, (c0, w) in zip(pss, grp):
                                    nc.tensor.matmul(
                                        ps[:, :w],
                                        wdt[:, d, fj, :], h[:, fj, c0:c0 + w],
                                        start=(fj == 0), stop=(fj == KF - 1),
                                    )
                            for ps, (c0, w) in zip(pss, grp):
                                nc.vector.tensor_copy(
                                    y_all[:, d, c0:c0 + w], ps[:, :w]
                                )
                                if last_e and d == KD - 1:
                                    # drain the final d-tile chunk-wise on
                                    # alternating queues (ACT is silu-free)
                                    y_eng = nc.scalar if c0 == 0 else nc.sync
                                    y_eng.dma_start(
                                        y_d[:, d, s0 + c0:s0 + c0 + w],
                                        y_all[:, d, c0:c0 + w],
                                    )
                        if last_e and d < KD - 1:
                            # stream the tail expert out per d-tile
                            nc.sync.dma_start(
                                y_d[:, d, s0:s0 + cap], y_all[:, d, :]
                            )
                    if not last_e:
                        nc.sync.dma_start(
                            y_d[:, :, s0:s0 + cap], y_all[:]
                        )

    return nc


# ---------------------------------------------------------------------------
# Host-side sharding / unsharding
# ---------------------------------------------------------------------------

def _prep_wgu(wg16, wu16):
    """two [D, FQ] fp16 -> [128, KF, 2, KD, 128] p-major."""
    def t(w):
        return w.reshape(KD, 128, KF, 128).transpose(1, 2, 0, 3)  # p,fj,k,f
    return np.ascontiguousarray(
        np.stack([t(wg16), t(wu16)], axis=2))       # p, fj, m, k, f


def _prep_wd(w16):
    """[FQ, D] fp16 -> [128, KD, KF, 128] p-major."""
    arr = w16.reshape(KF, 128, KD, 128)             # fj, p, d, dd
    return np.ascontiguousarray(arr.transpose(1, 2, 0, 3))


_prog_cache = {}


def get_program(caps):
    key = tuple(caps)
    if key not in _prog_cache:
        _prog_cache[key] = build_program(caps)
    return _prog_cache[key]


def prepare_in_maps(np_inputs):
    x = np.asarray(np_inputs["x"])
    B, S, _ = x.shape
    xf = np.asarray(x.reshape(-1, D), dtype=np.float16)
    idx = np.asarray(np_inputs["expert_idx"]).reshape(-1)
    w_gate = np.asarray(np_inputs["w_gate"], dtype=np.float16)
    w_up = np.asarray(np_inputs["w_up"], dtype=np.float16)
    w_down = np.asarray(np_inputs["w_down"], dtype=np.float16)

    order = np.argsort(idx, kind="stable")
    counts = np.bincount(idx, minlength=E).astype(np.int64)
    caps = [max(16, int(math.ceil(max(c, 1) / 8)) * 8) for c in counts]
    T = sum(caps)
    starts = np.zeros(E + 1, dtype=np.int64)
    np.cumsum(caps, out=starts[1:])
    cstarts = np.zeros(E + 1, dtype=np.int64)
    np.cumsum(counts, out=cstarts[1:])

    tok_of = [order[cstarts[e]:cstarts[e + 1]] for e in range(E)]

    # x packed [128, KD, T] p-major, per-expert segments padded to caps
    xt = np.zeros((D, T), dtype=np.float16)
    for e in range(E):
        xt[:, starts[e]:starts[e] + counts[e]] = xf[tok_of[e]].T
    x_arr = np.ascontiguousarray(
        xt.reshape(KD, 128, T).transpose(1, 0, 2))

    in_maps = []
    for q in range(E):
        sl = slice(q * FQ, (q + 1) * FQ)
        wgu = np.ascontiguousarray(np.stack(
            [_prep_wgu(w_gate[e][:, sl], w_up[e][:, sl]) for e in range(E)]))
        wd = np.ascontiguousarray(np.stack(
            [_prep_wd(w_down[e][sl, :]) for e in range(E)]))
        in_maps.append({"x": x_arr, "wgu": wgu, "wd": wd})
    meta = (tok_of, counts, starts, (B, S), x.dtype)
    return in_maps, caps, meta


def unshard(results, caps, meta):
    tok_of, counts, starts, (B, S), out_dtype = meta
    T = sum(caps)
    acc = np.zeros((128, KD, T), dtype=np.float32)
    for q in range(E):
        acc += np.asarray(results[q]["y"]).astype(np.float32)
    acc = acc.transpose(1, 0, 2).reshape(D, T)
    out = np.zeros((B * S, D), dtype=np.float32)
    for e in range(E):
        out[tok_of[e]] = acc[:, starts[e]:starts[e] + counts[e]].T
    return out.reshape(B, S, D).astype(out_dtype, copy=False)


def kernel(x, expert_idx, w_gate, w_up, w_down):
    np_inputs = {"x": x, "expert_idx": expert_idx, "w_gate": w_gate,
                 "w_up": w_up, "w_down": w_down}
    in_maps, caps, meta = prepare_in_maps(np_inputs)
    nc = get_program(caps)
    try:
        res = run_bass_kernel_spmd(nc, in_maps, list(range(8)))
    except Exception:
        res = run_bass_kernel_spmd(nc, in_maps, list(range(8)))
    return unshard(res.results, caps, meta)

